# revision 1
# baseline (speedup 1.0000x reference)
"""Cross-attention Trainium2 kernel, tensor-parallel over 8 NeuronCores.

Sharding: core c handles batch b = c // 4 and head-group hg = c % 4
(4 heads = 512 of the 2048 hidden dims). Each core computes its heads'
QKV projections, RoPE, softmax attention (transposed-scores layout with a
matmul-based denominator), and a partial output projection. The host sums
the 4 partials per batch.

Self-contained: hardcodes all shapes from the problem spec.
"""

import numpy as np
import ml_dtypes

import concourse.bacc as bacc
import concourse.tile as tile
from concourse import mybir
from concourse.bass_utils import run_bass_kernel_spmd

BF16 = ml_dtypes.bfloat16

B, T, S = 2, 2048, 2048
QD, CD = 2048, 2048
H, D = 16, 128
NE = 64            # rotary dims
KVMAX = 2048
N_CORES = 8
N_HG = 4           # head groups (cores per batch)
HPC = H // N_HG    # heads per core = 4
HD = HPC * D       # 512 head dims per core
KT = CD // 128     # contraction tiles = 16
ST = KVMAX // 128  # kv tiles = 16
CW = 512           # q chunk width
NCH = T // CW      # 4 chunks
SCALE = float(D) ** -0.5
IDENT32 = list(range(32))

DT_B = mybir.dt.bfloat16
DT_F = mybir.dt.float32

_compiled = {}


def _build(reps=1):
    if reps in _compiled:
        return _compiled[reps]

    nc = bacc.Bacc("TRN2", target_bir_lowering=False, debug=False,
                   num_devices=N_CORES)

    # All big operands arrive pre-tiled on the host so every DMA reads
    # >=4KB contiguous per partition: [partition, ...tile dims...]
    xT = nc.dram_tensor("xT", [128, NCH, KT, CW], DT_B, kind="ExternalInput")
    yT = nc.dram_tensor("yT", [128, KT, KVMAX], DT_B, kind="ExternalInput")
    wqT = nc.dram_tensor("wqT", [128, KT, HD], DT_B, kind="ExternalInput")
    wkT = nc.dram_tensor("wkT", [128, KT, HD], DT_B, kind="ExternalInput")
    wvT = nc.dram_tensor("wvT", [128, KT, HD], DT_B, kind="ExternalInput")
    woT = nc.dram_tensor("woT", [128, HPC, QD], DT_B, kind="ExternalInput")
    cosq = nc.dram_tensor("cosq", [NE, T], DT_B, kind="ExternalInput")
    sinq = nc.dram_tensor("sinq", [NE, T], DT_B, kind="ExternalInput")
    cosk = nc.dram_tensor("cosk", [NE, KVMAX], DT_B, kind="ExternalInput")
    sink = nc.dram_tensor("sink", [NE, KVMAX], DT_B, kind="ExternalInput")
    partial = nc.dram_tensor("partial", [T, QD], DT_F, kind="ExternalOutput")

    with tile.TileContext(nc) as tc:
        if reps == 1:
            _body(nc, tc, xT, yT, wqT, wkT, wvT, woT, cosq, sinq, cosk,
                  sink, partial)
        else:
            with tc.For_i(0, reps, 1):
                _body(nc, tc, xT, yT, wqT, wkT, wvT, woT, cosq, sinq,
                      cosk, sink, partial)

    nc.compile()
    _compiled[reps] = nc
    return nc


def _rope(nc, pool, dst, cos_sb, sin_sb, w):
    """In-place RoPE on dst[0:NE, :w] (head-dim on partitions).

    cos_sb/sin_sb are [NE, w] slices; sin rows 0:32 carry -sin, 32:64 +sin.
    """
    rot = pool.tile([NE, w], DT_B, tag="rot")
    half = NE // 2
    nc.vector.stream_shuffle(rot[0:half, :], dst[half:NE, :], IDENT32)
    nc.vector.stream_shuffle(rot[half:NE, :], dst[0:half, :], IDENT32)
    nc.vector.tensor_mul(rot[:, :], rot[:, :], sin_sb)
    nc.vector.tensor_mul(dst[0:NE, :], dst[0:NE, :], cos_sb)
    nc.vector.tensor_add(dst[0:NE, :], dst[0:NE, :], rot[:, :])


def _qproj(nc, qps, qpool, rope_pool, wq_sb, x_sb, cosq_sb, sinq_sb, c, h):
    """Q projection + RoPE for (chunk c, head h) -> q_sb [D, CW] bf16."""
    qp = qps.tile([128, CW], DT_F, tag="qp")
    for kk in range(KT):
        nc.tensor.matmul(
            qp[:], wq_sb[:, kk, h * 128:(h + 1) * 128], x_sb[:, kk, :],
            start=(kk == 0), stop=(kk == KT - 1))
    q_sb = qpool.tile([128, CW], DT_B, tag="qsb")
    nc.vector.tensor_copy(q_sb[:], qp[:])
    _rope(nc, rope_pool, q_sb[:],
          cosq_sb[:, c * CW:(c + 1) * CW],
          sinq_sb[:, c * CW:(c + 1) * CW], CW)
    return q_sb


def _body(nc, tc, xT, yT, wqT, wkT, wvT, woT, cosq, sinq, cosk, sink,
          partial):
    from contextlib import ExitStack

    with ExitStack() as ctx:
        const = ctx.enter_context(tc.tile_pool(name="const", bufs=1))
        wpool = ctx.enter_context(tc.tile_pool(name="w", bufs=1))
        kvpool = ctx.enter_context(tc.tile_pool(name="kv", bufs=1))
        rope_pool = ctx.enter_context(tc.tile_pool(name="rope", bufs=2))
        qpool = ctx.enter_context(tc.tile_pool(name="q", bufs=HPC + 2))
        qps = ctx.enter_context(tc.tile_pool(name="qps", bufs=1,
                                             space="PSUM"))

        # ---- Phase A: chunk-0 Q projections while K/V inputs stream ----
        wq_sb = wpool.tile([128, KT, HD], DT_B)
        wo_sb = wpool.tile([128, HPC, QD], DT_B)
        for g in range(4):
            nc.sync.dma_start(wq_sb[:, g * 4:(g + 1) * 4, :],
                              wqT.ap()[:, g * 4:(g + 1) * 4, :])
        for g in range(HPC):
            nc.sync.dma_start(wo_sb[:, g, :], woT.ap()[:, g, :])

        zbias = const.tile([128, 1], DT_F)
        nc.gpsimd.memset(zbias[:], 0.0)
        ones_col = const.tile([128, 1], DT_B)
        nc.gpsimd.memset(ones_col[:], 1.0)
        ones_row = const.tile([1, 128], DT_F)
        nc.gpsimd.memset(ones_row[:], 1.0)
        cosq_sb = const.tile([NE, T], DT_B)
        nc.sync.dma_start(cosq_sb[:], cosq.ap())
        sinq_sb = const.tile([NE, T], DT_B)
        nc.sync.dma_start(sinq_sb[:], sinq.ap())

        k_sb = kvpool.tile([128, HPC, KVMAX], DT_B)
        v_sb = kvpool.tile([128, ST, HD], DT_B)

        q_c0 = []
        with tc.tile_pool(name="xA", bufs=1) as xApool:
            xa_sb = xApool.tile([128, KT, CW], DT_B, tag="xa")
            for g in range(4):
                nc.sync.dma_start(xa_sb[:, g * 4:(g + 1) * 4, :],
                                  xT.ap()[:, 0, g * 4:(g + 1) * 4, :])
            for h in range(HPC):
                q_c0.append(_qproj(nc, qps, qpool, rope_pool, wq_sb,
                                   xa_sb, cosq_sb, sinq_sb, 0, h))

            # ---- Phase 1: K / V projections (+ RoPE on K) ----
            with (
                tc.tile_pool(name="wkv", bufs=1) as wkvpool,
                tc.tile_pool(name="y", bufs=1) as ypool,
                tc.tile_pool(name="ps1", bufs=6, space="PSUM") as ps1,
            ):
                cosk_sb = wkvpool.tile([NE, KVMAX], DT_B)
                nc.sync.dma_start(cosk_sb[:], cosk.ap())
                sink_sb = wkvpool.tile([NE, KVMAX], DT_B)
                nc.sync.dma_start(sink_sb[:], sink.ap())
                wk_sb = wkvpool.tile([128, KT, HD], DT_B)
                wv_sb = wkvpool.tile([128, KT, HD], DT_B)
                for g in range(4):
                    nc.sync.dma_start(wk_sb[:, g * 4:(g + 1) * 4, :],
                                      wkT.ap()[:, g * 4:(g + 1) * 4, :])
                    nc.sync.dma_start(wv_sb[:, g * 4:(g + 1) * 4, :],
                                      wvT.ap()[:, g * 4:(g + 1) * 4, :])

                y_sb = ypool.tile([128, KT, KVMAX], DT_B)
                for g in range(8):
                    nc.sync.dma_start(y_sb[:, g * 2:(g + 1) * 2, :],
                                      yT.ap()[:, g * 2:(g + 1) * 2, :])

                for st in range(ST):
                    vp = ps1.tile([128, HD], DT_F, tag="p1")
                    for kk in range(KT):
                        nc.tensor.matmul(
                            vp[:], y_sb[:, kk, st * 128:(st + 1) * 128],
                            wv_sb[:, kk, :],
                            start=(kk == 0), stop=(kk == KT - 1))
                    if st % 2 == 0:
                        nc.vector.tensor_copy(v_sb[:, st, :], vp[:])
                    else:
                        nc.scalar.copy(v_sb[:, st, :], vp[:])

                for h in range(HPC):
                    for sc in range(KVMAX // 512):
                        kp = ps1.tile([128, 512], DT_F, tag="p1")
                        for kk in range(KT):
                            nc.tensor.matmul(
                                kp[:], wk_sb[:, kk, h * 128:(h + 1) * 128],
                                y_sb[:, kk, sc * 512:(sc + 1) * 512],
                                start=(kk == 0), stop=(kk == KT - 1))
                        if sc % 2 == 0:
                            nc.vector.tensor_copy(
                                k_sb[:, h, sc * 512:(sc + 1) * 512], kp[:])
                        else:
                            nc.scalar.copy(
                                k_sb[:, h, sc * 512:(sc + 1) * 512], kp[:])
                    _rope(nc, rope_pool, k_sb[:, h, :], cosk_sb[:, :],
                          sink_sb[:, :], KVMAX)

        # ---- Phase 2: per q-chunk attention + output projection ----
        with ExitStack() as c2:
            xpool = c2.enter_context(tc.tile_pool(name="x", bufs=2))
            apool = c2.enter_context(tc.tile_pool(name="attn", bufs=6))
            spool = c2.enter_context(tc.tile_pool(name="psum8", bufs=3))
            opool = c2.enter_context(tc.tile_pool(name="o", bufs=2 * HPC))
            dpool = c2.enter_context(tc.tile_pool(name="den", bufs=2))
            ppool = c2.enter_context(tc.tile_pool(name="part", bufs=3))
            # PSUM (8 banks with outer qps=1): sps(2x2) avps(1) dfps(2:
            # denominator + broadcast + output-projection tiles)
            sps = c2.enter_context(tc.tile_pool(name="sps", bufs=2,
                                                space="PSUM"))
            avps = c2.enter_context(tc.tile_pool(name="avps", bufs=1,
                                                 space="PSUM"))
            dfps = c2.enter_context(tc.tile_pool(name="dfps", bufs=2,
                                                 space="PSUM"))

            def normalize(av, den):
                # o = av * (1/den), denominator broadcast over partitions
                # via a K=1 matmul
                den_sb = dpool.tile([1, CW], DT_F, tag="densb")
                nc.scalar.copy(den_sb[:], den[:])
                bc = dfps.tile([128, CW], DT_F, tag="fp")
                nc.tensor.matmul(bc[:], ones_row[:], den_sb[:],
                                 start=True, stop=True)
                bc_sb = dpool.tile([128, CW], DT_F, tag="bcsb")
                nc.vector.reciprocal(bc_sb[:], bc[:])
                o_sb = opool.tile([128, CW], DT_B, tag="osb")
                nc.vector.tensor_mul(o_sb[:], bc_sb[:], av[:])
                return o_sb

            for c in range(NCH):
                if c > 0:
                    x_sb = xpool.tile([128, KT, CW], DT_B, tag="x")
                    for g in range(4):
                        nc.sync.dma_start(x_sb[:, g * 4:(g + 1) * 4, :],
                                          xT.ap()[:, c, g * 4:(g + 1) * 4, :])

                o_tiles = []
                pending = None
                for h in range(HPC):
                    if c == 0:
                        q_sb = q_c0[h]
                    else:
                        q_sb = _qproj(nc, qps, qpool, rope_pool, wq_sb,
                                      x_sb, cosq_sb, sinq_sb, c, h)
                    # normalize the previous head here: its denominator
                    # copy (ACT) overlaps this head's q projection, so the
                    # broadcast matmul never stalls the PE
                    if pending is not None:
                        o_tiles.append(normalize(*pending))

                    # scoresT = K_T.T @ q -> [kv, q], two kv-tiles per PSUM
                    # pair; one exp per pair; pair-sum on DVE halves the
                    # denominator matmuls; attn@V rides along per tile
                    den = dfps.tile([1, CW], DT_F, tag="fp")
                    av = avps.tile([128, CW], DT_F, tag="av")
                    ps_prev = None
                    ps2_prev = None
                    for sg in range(ST // 2):
                        sp = sps.tile([128, 2, CW], DT_F, tag="sp")
                        for j in range(2):
                            st = 2 * sg + j
                            nc.tensor.matmul(
                                sp[:, j, :],
                                k_sb[:, h, st * 128:(st + 1) * 128],
                                q_sb[:], start=True, stop=True)
                        at = apool.tile([128, 2, CW], DT_B, tag="at")
                        nc.scalar.activation(
                            at[:, :, :], sp[:, :, :],
                            mybir.ActivationFunctionType.Exp, bias=zbias[:])
                        ps = spool.tile([128, CW], DT_B, tag="ps")
                        nc.vector.tensor_add(ps[:], at[:, 0, :], at[:, 1, :])
                        if sg % 2 == 1:
                            ps2 = spool.tile([128, CW], DT_B, tag="ps2")
                            nc.vector.tensor_add(ps2[:], ps_prev[:], ps[:])
                            if sg % 4 == 3:
                                ps4 = spool.tile([128, CW], DT_B, tag="ps4")
                                nc.vector.tensor_add(ps4[:], ps2_prev[:],
                                                     ps2[:])
                                nc.tensor.matmul(
                                    den[:], ones_col[:], ps4[:],
                                    start=(sg == 3),
                                    stop=(sg == ST // 2 - 1))
                            ps2_prev = ps2
                        ps_prev = ps
                        for j in range(2):
                            st = 2 * sg + j
                            nc.tensor.matmul(
                                av[:], v_sb[:, st, h * 128:(h + 1) * 128],
                                at[:, j, :], start=(st == 0),
                                stop=(st == ST - 1))
                    pending = (av, den)
                o_tiles.append(normalize(*pending))

                # output projection: partial[qt, :] = sum_h o_h.T @ woT_h
                for qt in range(CW // 128):
                    part_sb = ppool.tile([128, QD], DT_F, tag="part")
                    for nt in range(QD // 512):
                        fp = dfps.tile([128, 512], DT_F, tag="fp")
                        for h in range(HPC):
                            nc.tensor.matmul(
                                fp[:],
                                o_tiles[h][:, qt * 128:(qt + 1) * 128],
                                wo_sb[:, h, nt * 512:(nt + 1) * 512],
                                start=(h == 0), stop=(h == HPC - 1))
                        nc.vector.tensor_copy(
                            part_sb[:, nt * 512:(nt + 1) * 512], fp[:])
                    row0 = c * CW + qt * 128
                    nc.sync.dma_start(partial[row0:row0 + 128, :],
                                      part_sb[:])


def _tile_rows(a, p=128):
    """[R, M] with R = n*p  ->  [p, n, M] (partition-major tiling)."""
    r, m = a.shape
    return np.ascontiguousarray(
        a.reshape(r // p, p, m).transpose(1, 0, 2))


def _host_shards(inputs):
    """Build the 8 per-core input maps from the full inputs."""
    x = np.asarray(inputs["x"], np.float32)
    y = np.asarray(inputs["y"], np.float32)
    rope_cos = np.asarray(inputs["rope_cos"], np.float32)
    rope_sin = np.asarray(inputs["rope_sin"], np.float32)
    wq = np.asarray(inputs["wq"], np.float32)
    wk = np.asarray(inputs["wk"], np.float32)
    wv = np.asarray(inputs["wv"], np.float32)
    wo = np.asarray(inputs["wo"], np.float32)
    input_pos = np.asarray(inputs["input_pos"], np.int64)

    # KV-cache scatter folded into a host-side permutation of y's rows and
    # of the rope tables (k positions live at cache slot input_pos[s]).
    y_cache = np.zeros((B, KVMAX, CD), np.float32)
    y_cache[:, input_pos, :] = y
    ck = np.zeros((KVMAX, NE // 2), np.float32)
    ck[input_pos] = rope_cos
    sk = np.zeros((KVMAX, NE // 2), np.float32)
    sk[input_pos] = rope_sin

    def tabT(cos2, sin2):
        cosT = np.tile(cos2.T, (2, 1)).astype(BF16)          # [NE, S]
        sinT = np.concatenate([-sin2.T, sin2.T], 0).astype(BF16)
        return np.ascontiguousarray(cosT), np.ascontiguousarray(sinT)

    cosq_h, sinq_h = tabT(rope_cos[:T], rope_sin[:T])
    cosk_h, sink_h = tabT(ck, sk)

    in_maps = []
    for core in range(N_CORES):
        b, hg = core // N_HG, core % N_HG
        rows = slice(hg * HD, (hg + 1) * HD)
        xt = _tile_rows(x[b].T.astype(BF16))        # [128, KT, T]
        xt = np.ascontiguousarray(
            xt.reshape(128, KT, NCH, CW).transpose(0, 2, 1, 3))
        in_maps.append({
            "xT": xt,                               # [128, NCH, KT, CW]
            "yT": _tile_rows(y_cache[b].T.astype(BF16)),
            "wqT": _tile_rows((wq[rows] * SCALE).T.astype(BF16)),
            "wkT": _tile_rows(wk[rows].T.astype(BF16)),
            "wvT": _tile_rows(wv[rows].T.astype(BF16)),
            "woT": _tile_rows(wo[:, rows].T.astype(BF16)),
            "cosq": cosq_h, "sinq": sinq_h,
            "cosk": cosk_h, "sink": sink_h,
        })
    return in_maps


def _run(inputs, trace=False, reps=1, **kw):
    nc = _build(reps)
    in_maps = _host_shards(inputs)
    res = run_bass_kernel_spmd(nc, in_maps, list(range(N_CORES)),
                               trace=trace, **kw)
    out = np.zeros((B, T, QD), np.float32)
    for core in range(N_CORES):
        out[core // N_HG] += res.results[core]["partial"]
    return out, res


def kernel(**inputs):
    out, _ = _run(inputs)
    return out



# revision 9
# speedup vs baseline: 1.8104x; 1.8104x over previous
"""Cross-attention Trainium2 kernel, tensor-parallel over 8 NeuronCores.

Sharding: core c handles batch b = c // 4 and head-group hg = c % 4
(4 heads = 512 of the 2048 hidden dims). Each core computes its heads'
QKV projections, RoPE, softmax attention (transposed-scores layout), and
a partial output projection. The host sums the 4 partials per batch.

v2 schedule (PE-roofline oriented):
  P0  all Q projections + Q RoPE, overlapped with every input DMA
  P1  V projection (PSUM drains on ACT, which is otherwise idle)
  P2  per head h: attention over the 4 q-chunks with the NEXT head's
      K-projection matmuls interleaved into the exp-paced gaps; softmax
      denominator via DVE pair-sum tree + GPSIMD partition_all_reduce
      (no PE denominator/broadcast matmuls), 1/den via
      reciprocal_approx_fast
  P3  output projection, drains split ACT/DVE, bf16 partial DMA

Self-contained: hardcodes all shapes from the problem spec.
"""

import numpy as np
import ml_dtypes

import concourse.bacc as bacc
import concourse.bass_isa as bass_isa
import concourse.tile as tile
from concourse import mybir
from concourse.bass_utils import run_bass_kernel_spmd

BF16 = ml_dtypes.bfloat16

B, T, S = 2, 2048, 2048
QD, CD = 2048, 2048
H, D = 16, 128
NE = 64            # rotary dims
KVMAX = 2048
N_CORES = 8
N_HG = 4           # head groups (cores per batch)
HPC = H // N_HG    # heads per core = 4
HD = HPC * D       # 512 head dims per core
KT = CD // 128     # contraction tiles = 16
ST = KVMAX // 128  # kv tiles = 16
CW = 512           # q chunk width
NCH = T // CW      # 4 chunks
SCALE = float(D) ** -0.5
IDENT32 = list(range(32))

DT_B = mybir.dt.bfloat16
DT_F = mybir.dt.float32

_compiled = {}


def _build(reps=1):
    if reps in _compiled:
        return _compiled[reps]

    nc = bacc.Bacc("TRN2", target_bir_lowering=False, debug=False,
                   num_devices=N_CORES)

    xT = nc.dram_tensor("xT", [128, NCH, KT, CW], DT_B, kind="ExternalInput")
    yT = nc.dram_tensor("yT", [128, KT, KVMAX], DT_B, kind="ExternalInput")
    wqT = nc.dram_tensor("wqT", [128, KT, HD], DT_B, kind="ExternalInput")
    wkT = nc.dram_tensor("wkT", [128, KT, HD], DT_B, kind="ExternalInput")
    wvT = nc.dram_tensor("wvT", [128, KT, HD], DT_B, kind="ExternalInput")
    woT = nc.dram_tensor("woT", [128, HPC, QD], DT_B, kind="ExternalInput")
    cosq = nc.dram_tensor("cosq", [NE, T], DT_B, kind="ExternalInput")
    sinq = nc.dram_tensor("sinq", [NE, T], DT_B, kind="ExternalInput")
    cosk = nc.dram_tensor("cosk", [NE, KVMAX], DT_B, kind="ExternalInput")
    sink = nc.dram_tensor("sink", [NE, KVMAX], DT_B, kind="ExternalInput")
    partial = nc.dram_tensor("partial", [T, QD], DT_B, kind="ExternalOutput")

    with tile.TileContext(nc) as tc:
        if reps == 1:
            _body(nc, tc, xT, yT, wqT, wkT, wvT, woT, cosq, sinq, cosk,
                  sink, partial)
        else:
            with tc.For_i(0, reps, 1):
                _body(nc, tc, xT, yT, wqT, wkT, wvT, woT, cosq, sinq,
                      cosk, sink, partial)

    nc.compile()
    _compiled[reps] = nc
    return nc


def _rope(nc, pool, dst, cos_sb, sin_sb, w):
    """In-place RoPE on dst[0:NE, :w] (head-dim on partitions).

    cos_sb/sin_sb are [NE, w] slices; sin rows 0:32 carry -sin, 32:64 +sin.
    """
    rot = pool.tile([NE, w], DT_B, tag="rot")
    half = NE // 2
    nc.vector.stream_shuffle(rot[0:half, :], dst[half:NE, :], IDENT32)
    nc.vector.stream_shuffle(rot[half:NE, :], dst[0:half, :], IDENT32)
    nc.vector.tensor_mul(rot[:, :], rot[:, :], sin_sb)
    nc.vector.tensor_mul(dst[0:NE, :], dst[0:NE, :], cos_sb)
    nc.vector.tensor_add(dst[0:NE, :], dst[0:NE, :], rot[:, :])


def _body(nc, tc, xT, yT, wqT, wkT, wvT, woT, cosq, sinq, cosk, sink,
          partial):
    from contextlib import ExitStack

    with ExitStack() as ctx:
        const = ctx.enter_context(tc.tile_pool(name="const", bufs=1))
        qpool = ctx.enter_context(tc.tile_pool(name="q", bufs=1))
        kvpool = ctx.enter_context(tc.tile_pool(name="kv", bufs=1))
        ktab = ctx.enter_context(tc.tile_pool(name="ktab", bufs=1))
        rope_pool = ctx.enter_context(tc.tile_pool(name="rope", bufs=2))
        # PSUM: pps 2 + sps 2x2 + avps 2 = 8 banks
        pps = ctx.enter_context(tc.tile_pool(name="pps", bufs=2,
                                             space="PSUM"))
        sps = ctx.enter_context(tc.tile_pool(name="sps", bufs=2,
                                             space="PSUM"))
        avps = ctx.enter_context(tc.tile_pool(name="avps", bufs=2,
                                              space="PSUM"))

        zbias = const.tile([128, 1], DT_F)
        nc.gpsimd.memset(zbias[:], 0.0)

        q_all = qpool.tile([128, HPC, T], DT_B)
        k_sb = kvpool.tile([128, HPC, KVMAX], DT_B)
        v_sb = kvpool.tile([128, ST, HD], DT_B)
        cosk_sb = ktab.tile([NE, KVMAX], DT_B)
        sink_sb = ktab.tile([NE, KVMAX], DT_B)
        nc.sync.dma_start(cosk_sb[:], cosk.ap())
        nc.sync.dma_start(sink_sb[:], sink.ap())

        # ---- P0: all Q projections (+ RoPE) while x streams ----
        with ExitStack() as c0:
            wqpool = c0.enter_context(tc.tile_pool(name="wq", bufs=1))
            xpool = c0.enter_context(tc.tile_pool(name="x", bufs=2))
            qtab = c0.enter_context(tc.tile_pool(name="qtab", bufs=1))

            wq_sb = wqpool.tile([128, KT, HD], DT_B)
            for g in range(4):
                nc.sync.dma_start(wq_sb[:, g * 4:(g + 1) * 4, :],
                                  wqT.ap()[:, g * 4:(g + 1) * 4, :])
            cosq_sb = qtab.tile([NE, T], DT_B)
            nc.sync.dma_start(cosq_sb[:], cosq.ap())
            sinq_sb = qtab.tile([NE, T], DT_B)
            nc.sync.dma_start(sinq_sb[:], sinq.ap())

            x_tiles = []
            for c in range(NCH):
                x_sb = xpool.tile([128, KT, CW], DT_B, tag="x")
                for g in range(4):
                    nc.sync.dma_start(x_sb[:, g * 4:(g + 1) * 4, :],
                                      xT.ap()[:, c, g * 4:(g + 1) * 4, :])
                x_tiles.append(x_sb)

            for ci in range(NCH):
                x_sb = x_tiles[ci]
                for h in range(HPC):
                    qp = pps.tile([128, CW], DT_F, tag="pp")
                    for kk in range(KT):
                        nc.tensor.matmul(
                            qp[:], wq_sb[:, kk, h * 128:(h + 1) * 128],
                            x_sb[:, kk, :],
                            start=(kk == 0), stop=(kk == KT - 1))
                    dst = q_all[:, h, ci * CW:(ci + 1) * CW]
                    if h % 2 == 0:
                        nc.vector.tensor_copy(dst, qp[:])
                    else:
                        nc.scalar.copy(dst, qp[:])
                    _rope(nc, rope_pool, dst,
                          cosq_sb[:, ci * CW:(ci + 1) * CW],
                          sinq_sb[:, ci * CW:(ci + 1) * CW], CW)

        # y / wk / wv / wo loads issued behind the P0 x DMAs
        ypool = ctx.enter_context(tc.tile_pool(name="y", bufs=1))
        wkpool = ctx.enter_context(tc.tile_pool(name="wk", bufs=1))
        y_sb = ypool.tile([128, KT, KVMAX], DT_B)
        for g in range(8):
            nc.sync.dma_start(y_sb[:, g * 2:(g + 1) * 2, :],
                              yT.ap()[:, g * 2:(g + 1) * 2, :])
        wk_sb = wkpool.tile([128, KT, HD], DT_B)
        for g in range(4):
            nc.sync.dma_start(wk_sb[:, g * 4:(g + 1) * 4, :],
                              wkT.ap()[:, g * 4:(g + 1) * 4, :])

        with ExitStack() as c1:
            wvpool = c1.enter_context(tc.tile_pool(name="wv", bufs=1))
            wv_sb = wvpool.tile([128, KT, HD], DT_B)
            for g in range(4):
                nc.sync.dma_start(wv_sb[:, g * 4:(g + 1) * 4, :],
                                  wvT.ap()[:, g * 4:(g + 1) * 4, :])

            # ---- P1: V projection; drains on ACT (idle here) ----
            for st in range(ST):
                vp = pps.tile([128, HD], DT_F, tag="pp")
                for kk in range(KT):
                    nc.tensor.matmul(
                        vp[:], y_sb[:, kk, st * 128:(st + 1) * 128],
                        wv_sb[:, kk, :],
                        start=(kk == 0), stop=(kk == KT - 1))
                nc.scalar.copy(v_sb[:, st, :], vp[:])

        # ---- P2: per-head attention with next head's K proj woven in ----
        o_tiles = {}
        opool = ctx.enter_context(tc.tile_pool(name="o", bufs=16))
        wopool = ctx.enter_context(tc.tile_pool(name="wo", bufs=1))
        wo_sb = wopool.tile([128, HPC, QD], DT_B)
        for g in range(HPC):
            nc.sync.dma_start(wo_sb[:, g, :], woT.ap()[:, g, :])
        with ExitStack() as c2:
            apool = c2.enter_context(tc.tile_pool(name="attn", bufs=4))
            lpool = c2.enter_context(tc.tile_pool(name="leaf", bufs=3))
            p2pool = c2.enter_context(tc.tile_pool(name="p2", bufs=3))
            p4pool = c2.enter_context(tc.tile_pool(name="p4", bufs=2))
            dpool = c2.enter_context(tc.tile_pool(name="den", bufs=1))

            def kproj_group(h, sc, kk_lo, kk_hi, kp):
                """Emit K-proj matmuls kk_lo..kk_hi for (head h, chunk sc)."""
                for kk in range(kk_lo, kk_hi):
                    nc.tensor.matmul(
                        kp[:], wk_sb[:, kk, h * 128:(h + 1) * 128],
                        y_sb[:, kk, sc * CW:(sc + 1) * CW],
                        start=(kk == 0), stop=(kk == KT - 1))

            def kproj_finish(h, sc, kp):
                dst = k_sb[:, h, sc * CW:(sc + 1) * CW]
                nc.vector.tensor_copy(dst, kp[:])
                _rope(nc, rope_pool, dst,
                      cosk_sb[:, sc * CW:(sc + 1) * CW],
                      sink_sb[:, sc * CW:(sc + 1) * CW], CW)

            # K proj head 0 upfront (drains on DVE; ACT idle)
            for sc in range(NCH):
                kp = pps.tile([128, CW], DT_F, tag="pp")
                kproj_group(0, sc, 0, KT, kp)
                kproj_finish(0, sc, kp)

            for h in range(HPC):
                for c in range(NCH):
                    # interleaved K proj (h+1, chunk c): 16 MMs woven into
                    # this attention pass, 2 per exp-paced gap
                    ikp = None
                    if h + 1 < HPC:
                        ikp = pps.tile([128, CW], DT_F, tag="pp")

                    q_ap = q_all[:, h, c * CW:(c + 1) * CW]
                    av = avps.tile([128, CW], DT_F, tag="av")
                    leaves = []
                    at_prev = None
                    for g in range(ST // 2):
                        sp = sps.tile([128, 2, CW], DT_F, tag="sp")
                        for j in range(2):
                            st = 2 * g + j
                            nc.tensor.matmul(
                                sp[:, j, :],
                                k_sb[:, h, st * 128:(st + 1) * 128],
                                q_ap, start=True, stop=True)
                        if ikp is not None:
                            kproj_group(h + 1, c, 2 * g, 2 * g + 2, ikp)
                        if at_prev is not None:
                            for j in range(2):
                                st = 2 * (g - 1) + j
                                nc.tensor.matmul(
                                    av[:],
                                    v_sb[:, st, h * 128:(h + 1) * 128],
                                    at_prev[:, j, :], start=(st == 0),
                                    stop=False)
                        at = apool.tile([128, 2, CW], DT_B, tag="at")
                        nc.scalar.activation(
                            at[:, :, :], sp[:, :, :],
                            mybir.ActivationFunctionType.Exp, bias=zbias[:])
                        ps = lpool.tile([128, CW], DT_B, tag="ps")
                        nc.vector.tensor_add(ps[:], at[:, 0, :], at[:, 1, :])
                        leaves.append(ps)
                        at_prev = at
                    # tail: av for the last pair
                    for j in range(2):
                        st = ST - 2 + j
                        nc.tensor.matmul(
                            av[:], v_sb[:, st, h * 128:(h + 1) * 128],
                            at_prev[:, j, :], start=False,
                            stop=(st == ST - 1))
                    if ikp is not None:
                        kproj_finish(h + 1, c, ikp)

                    # denominator: pair-sum tree (ps2 on GPSIMD, rest DVE),
                    # then partition all-reduce + approx reciprocal
                    ps2 = []
                    for i in range(4):
                        t = p2pool.tile([128, CW], DT_B, tag="p2")
                        nc.gpsimd.tensor_add(t[:], leaves[2 * i][:],
                                             leaves[2 * i + 1][:])
                        ps2.append(t)
                    p4a = p4pool.tile([128, CW], DT_B, tag="p4")
                    nc.vector.tensor_add(p4a[:], ps2[0][:], ps2[1][:])
                    p4b = p4pool.tile([128, CW], DT_B, tag="p4")
                    nc.vector.tensor_add(p4b[:], ps2[2][:], ps2[3][:])
                    den = dpool.tile([128, CW], DT_F, tag="den")
                    nc.vector.tensor_add(den[:], p4a[:], p4b[:])
                    denr = dpool.tile([128, CW], DT_F, tag="denr")
                    nc.gpsimd.partition_all_reduce(
                        denr[:], den[:], channels=128,
                        reduce_op=bass_isa.ReduceOp.add)
                    nc.vector.reciprocal_approx_fast(denr[:], denr[:])
                    o_sb = opool.tile([128, CW], DT_B, tag="o")
                    nc.vector.tensor_mul(o_sb[:], denr[:], av[:])
                    o_tiles[(c, h)] = o_sb

        # ---- P3: output projection; drains split ACT/DVE; bf16 DMA ----
        with tc.tile_pool(name="part", bufs=3) as ppart:
            for c in range(NCH):
                for qt in range(CW // 128):
                    part_sb = ppart.tile([128, QD], DT_B, tag="part")
                    for nt in range(QD // 512):
                        fp = pps.tile([128, 512], DT_F, tag="pp")
                        for h in range(HPC):
                            nc.tensor.matmul(
                                fp[:],
                                o_tiles[(c, h)][:, qt * 128:(qt + 1) * 128],
                                wo_sb[:, h, nt * 512:(nt + 1) * 512],
                                start=(h == 0), stop=(h == HPC - 1))
                        if nt % 2 == 0:
                            nc.vector.tensor_copy(
                                part_sb[:, nt * 512:(nt + 1) * 512], fp[:])
                        else:
                            nc.scalar.copy(
                                part_sb[:, nt * 512:(nt + 1) * 512], fp[:])
                    row0 = c * CW + qt * 128
                    nc.sync.dma_start(partial[row0:row0 + 128, :],
                                      part_sb[:])


def _tile_rows(a, p=128):
    """[R, M] with R = n*p  ->  [p, n, M] (partition-major tiling)."""
    r, m = a.shape
    return np.ascontiguousarray(
        a.reshape(r // p, p, m).transpose(1, 0, 2))


def _host_shards(inputs):
    """Build the 8 per-core input maps from the full inputs."""
    x = np.asarray(inputs["x"], np.float32)
    y = np.asarray(inputs["y"], np.float32)
    rope_cos = np.asarray(inputs["rope_cos"], np.float32)
    rope_sin = np.asarray(inputs["rope_sin"], np.float32)
    wq = np.asarray(inputs["wq"], np.float32)
    wk = np.asarray(inputs["wk"], np.float32)
    wv = np.asarray(inputs["wv"], np.float32)
    wo = np.asarray(inputs["wo"], np.float32)
    input_pos = np.asarray(inputs["input_pos"], np.int64)

    # KV-cache scatter folded into a host-side permutation of y's rows and
    # of the rope tables (k positions live at cache slot input_pos[s]).
    y_cache = np.zeros((B, KVMAX, CD), np.float32)
    y_cache[:, input_pos, :] = y
    ck = np.zeros((KVMAX, NE // 2), np.float32)
    ck[input_pos] = rope_cos
    sk = np.zeros((KVMAX, NE // 2), np.float32)
    sk[input_pos] = rope_sin

    def tabT(cos2, sin2):
        cosT = np.tile(cos2.T, (2, 1)).astype(BF16)          # [NE, S]
        sinT = np.concatenate([-sin2.T, sin2.T], 0).astype(BF16)
        return np.ascontiguousarray(cosT), np.ascontiguousarray(sinT)

    cosq_h, sinq_h = tabT(rope_cos[:T], rope_sin[:T])
    cosk_h, sink_h = tabT(ck, sk)

    in_maps = []
    for core in range(N_CORES):
        b, hg = core // N_HG, core % N_HG
        rows = slice(hg * HD, (hg + 1) * HD)
        xt = _tile_rows(x[b].T.astype(BF16))        # [128, KT, T]
        xt = np.ascontiguousarray(
            xt.reshape(128, KT, NCH, CW).transpose(0, 2, 1, 3))
        in_maps.append({
            "xT": xt,                               # [128, NCH, KT, CW]
            "yT": _tile_rows(y_cache[b].T.astype(BF16)),
            "wqT": _tile_rows((wq[rows] * SCALE).T.astype(BF16)),
            "wkT": _tile_rows(wk[rows].T.astype(BF16)),
            "wvT": _tile_rows(wv[rows].T.astype(BF16)),
            "woT": _tile_rows(wo[:, rows].T.astype(BF16)),
            "cosq": cosq_h, "sinq": sinq_h,
            "cosk": cosk_h, "sink": sink_h,
        })
    return in_maps


def _run(inputs, trace=False, reps=1, **kw):
    nc = _build(reps)
    in_maps = _host_shards(inputs)
    res = run_bass_kernel_spmd(nc, in_maps, list(range(N_CORES)),
                               trace=trace, **kw)
    out = np.zeros((B, T, QD), np.float32)
    for core in range(N_CORES):
        out[core // N_HG] += np.asarray(res.results[core]["partial"],
                                        dtype=np.float32)
    return out, res


def kernel(**inputs):
    out, _ = _run(inputs)
    return out


# revision 13
# speedup vs baseline: 1.9380x; 1.0705x over previous
"""Cross-attention Trainium2 kernel, tensor-parallel over 8 NeuronCores.

Sharding: core c handles batch b = c // 4 and head-group hg = c % 4
(4 heads = 512 of the 2048 hidden dims). Each core computes its heads'
QKV projections, RoPE, softmax attention (transposed-scores layout), and
a partial output projection. The host sums the 4 partials per batch.

v2 schedule (PE-roofline oriented):
  P0  all Q projections + Q RoPE, overlapped with every input DMA
  P1  V projection (PSUM drains on ACT, which is otherwise idle)
  P2  per head h: attention over the 4 q-chunks with the NEXT head's
      K-projection matmuls interleaved into the exp-paced gaps; softmax
      denominator via DVE pair-sum tree + GPSIMD partition_all_reduce
      (no PE denominator/broadcast matmuls), 1/den via
      reciprocal_approx_fast
  P3  output projection, drains split ACT/DVE, bf16 partial DMA

Self-contained: hardcodes all shapes from the problem spec.
"""

import numpy as np
import ml_dtypes

import concourse.bacc as bacc
import concourse.bass_isa as bass_isa
import concourse.tile as tile
from concourse import mybir
from concourse.bass_utils import run_bass_kernel_spmd

BF16 = ml_dtypes.bfloat16

B, T, S = 2, 2048, 2048
QD, CD = 2048, 2048
H, D = 16, 128
NE = 64            # rotary dims
KVMAX = 2048
N_CORES = 8
N_HG = 4           # head groups (cores per batch)
HPC = H // N_HG    # heads per core = 4
HD = HPC * D       # 512 head dims per core
KT = CD // 128     # contraction tiles = 16
ST = KVMAX // 128  # kv tiles = 16
CW = 512           # q chunk width
NCH = T // CW      # 4 chunks
SCALE = float(D) ** -0.5
IDENT32 = list(range(32))

DT_B = mybir.dt.bfloat16
DT_F = mybir.dt.float32

_compiled = {}


def _build(reps=1):
    if reps in _compiled:
        return _compiled[reps]

    nc = bacc.Bacc("TRN2", target_bir_lowering=False, debug=False,
                   num_devices=N_CORES)

    xT = nc.dram_tensor("xT", [128, NCH, KT, CW], DT_B, kind="ExternalInput")
    yT = nc.dram_tensor("yT", [128, KT, KVMAX], DT_B, kind="ExternalInput")
    wqT = nc.dram_tensor("wqT", [128, KT, HD], DT_B, kind="ExternalInput")
    wkT = nc.dram_tensor("wkT", [128, KT, HD], DT_B, kind="ExternalInput")
    wvT = nc.dram_tensor("wvT", [128, KT, HD], DT_B, kind="ExternalInput")
    woT = nc.dram_tensor("woT", [128, HPC, QD], DT_B, kind="ExternalInput")
    cosq = nc.dram_tensor("cosq", [NE, T], DT_B, kind="ExternalInput")
    sinq = nc.dram_tensor("sinq", [NE, T], DT_B, kind="ExternalInput")
    cosk = nc.dram_tensor("cosk", [NE, KVMAX], DT_B, kind="ExternalInput")
    sink = nc.dram_tensor("sink", [NE, KVMAX], DT_B, kind="ExternalInput")
    partial = nc.dram_tensor("partial", [T, QD], DT_B, kind="ExternalOutput")

    with tile.TileContext(nc) as tc:
        if reps == 1:
            _body(nc, tc, xT, yT, wqT, wkT, wvT, woT, cosq, sinq, cosk,
                  sink, partial)
        else:
            with tc.For_i(0, reps, 1):
                _body(nc, tc, xT, yT, wqT, wkT, wvT, woT, cosq, sinq,
                      cosk, sink, partial)

    nc.compile()
    _compiled[reps] = nc
    return nc


def _rope(nc, pool, dst, cos_sb, sin_sb, w):
    """In-place RoPE on dst[0:NE, :w] (head-dim on partitions).

    cos_sb/sin_sb are [NE, w] slices; sin rows 0:32 carry -sin, 32:64 +sin.
    """
    rot = pool.tile([NE, w], DT_B, tag="rot")
    half = NE // 2
    nc.vector.stream_shuffle(rot[0:half, :], dst[half:NE, :], IDENT32)
    nc.vector.stream_shuffle(rot[half:NE, :], dst[0:half, :], IDENT32)
    nc.vector.tensor_mul(rot[:, :], rot[:, :], sin_sb)
    nc.vector.tensor_mul(dst[0:NE, :], dst[0:NE, :], cos_sb)
    nc.vector.tensor_add(dst[0:NE, :], dst[0:NE, :], rot[:, :])


def _body(nc, tc, xT, yT, wqT, wkT, wvT, woT, cosq, sinq, cosk, sink,
          partial):
    from contextlib import ExitStack

    with ExitStack() as ctx:
        const = ctx.enter_context(tc.tile_pool(name="const", bufs=1))
        qpool = ctx.enter_context(tc.tile_pool(name="q", bufs=1))
        kvpool = ctx.enter_context(tc.tile_pool(name="kv", bufs=1))
        ktab = ctx.enter_context(tc.tile_pool(name="ktab", bufs=1))
        rope_pool = ctx.enter_context(tc.tile_pool(name="rope", bufs=2))
        # PSUM: pps 2 + sps 2x2 + avps 2 = 8 banks
        pps = ctx.enter_context(tc.tile_pool(name="pps", bufs=2,
                                             space="PSUM"))
        sps = ctx.enter_context(tc.tile_pool(name="sps", bufs=2,
                                             space="PSUM"))
        avps = ctx.enter_context(tc.tile_pool(name="avps", bufs=2,
                                              space="PSUM"))

        zbias = const.tile([128, 1], DT_F)
        nc.gpsimd.memset(zbias[:], 0.0)
        ones_col = const.tile([128, 1], DT_B)
        nc.gpsimd.memset(ones_col[:], 1.0)

        q_all = qpool.tile([128, HPC, T], DT_B)
        k_sb = kvpool.tile([128, HPC, KVMAX], DT_B)
        v_sb = kvpool.tile([128, ST, HD], DT_B)
        cosk_sb = ktab.tile([NE, KVMAX], DT_B)
        sink_sb = ktab.tile([NE, KVMAX], DT_B)
        nc.sync.dma_start(cosk_sb[:], cosk.ap())
        nc.sync.dma_start(sink_sb[:], sink.ap())

        # ---- P0: all Q projections (+ RoPE) while x streams ----
        with ExitStack() as c0:
            wqpool = c0.enter_context(tc.tile_pool(name="wq", bufs=1))
            xpool = c0.enter_context(tc.tile_pool(name="x", bufs=2))
            qtab = c0.enter_context(tc.tile_pool(name="qtab", bufs=1))

            wq_sb = wqpool.tile([128, KT, HD], DT_B)
            for g in range(4):
                nc.sync.dma_start(wq_sb[:, g * 4:(g + 1) * 4, :],
                                  wqT.ap()[:, g * 4:(g + 1) * 4, :])
            cosq_sb = qtab.tile([NE, T], DT_B)
            nc.sync.dma_start(cosq_sb[:], cosq.ap())
            sinq_sb = qtab.tile([NE, T], DT_B)
            nc.sync.dma_start(sinq_sb[:], sinq.ap())

            x_tiles = []
            for c in range(NCH):
                x_sb = xpool.tile([128, KT, CW], DT_B, tag="x")
                for g in range(4):
                    nc.sync.dma_start(x_sb[:, g * 4:(g + 1) * 4, :],
                                      xT.ap()[:, c, g * 4:(g + 1) * 4, :])
                x_tiles.append(x_sb)

            for ci in range(NCH):
                x_sb = x_tiles[ci]
                for h in range(HPC):
                    qp = pps.tile([128, CW], DT_F, tag="pp")
                    for kk in range(KT):
                        nc.tensor.matmul(
                            qp[:], wq_sb[:, kk, h * 128:(h + 1) * 128],
                            x_sb[:, kk, :],
                            start=(kk == 0), stop=(kk == KT - 1))
                    dst = q_all[:, h, ci * CW:(ci + 1) * CW]
                    if h % 2 == 0:
                        nc.vector.tensor_copy(dst, qp[:])
                    else:
                        nc.scalar.copy(dst, qp[:])
                    _rope(nc, rope_pool, dst,
                          cosq_sb[:, ci * CW:(ci + 1) * CW],
                          sinq_sb[:, ci * CW:(ci + 1) * CW], CW)

        # y / wk / wv / wo loads issued behind the P0 x DMAs
        ypool = ctx.enter_context(tc.tile_pool(name="y", bufs=1))
        wkpool = ctx.enter_context(tc.tile_pool(name="wk", bufs=1))
        y_sb = ypool.tile([128, KT, KVMAX], DT_B)
        for g in range(8):
            nc.sync.dma_start(y_sb[:, g * 2:(g + 1) * 2, :],
                              yT.ap()[:, g * 2:(g + 1) * 2, :])
        wk_sb = wkpool.tile([128, KT, HD], DT_B)
        for g in range(4):
            nc.sync.dma_start(wk_sb[:, g * 4:(g + 1) * 4, :],
                              wkT.ap()[:, g * 4:(g + 1) * 4, :])

        with ExitStack() as c1:
            wvpool = c1.enter_context(tc.tile_pool(name="wv", bufs=1))
            wv_sb = wvpool.tile([128, KT, HD], DT_B)
            for g in range(4):
                nc.sync.dma_start(wv_sb[:, g * 4:(g + 1) * 4, :],
                                  wvT.ap()[:, g * 4:(g + 1) * 4, :])

            # ---- P1: V projection; drains on ACT (idle here) ----
            for st in range(ST):
                vp = pps.tile([128, HD], DT_F, tag="pp")
                for kk in range(KT):
                    nc.tensor.matmul(
                        vp[:], y_sb[:, kk, st * 128:(st + 1) * 128],
                        wv_sb[:, kk, :],
                        start=(kk == 0), stop=(kk == KT - 1))
                nc.scalar.copy(v_sb[:, st, :], vp[:])

        # ---- P2: per-head attention with next head's K proj woven in ----
        o_tiles = {}
        opool = ctx.enter_context(tc.tile_pool(name="o", bufs=16))
        wopool = ctx.enter_context(tc.tile_pool(name="wo", bufs=1))
        wo_sb = wopool.tile([128, HPC, QD], DT_B)
        for g in range(HPC):
            nc.sync.dma_start(wo_sb[:, g, :], woT.ap()[:, g, :])
        with ExitStack() as c2:
            apool = c2.enter_context(tc.tile_pool(name="attn", bufs=3))
            lpool = c2.enter_context(tc.tile_pool(name="leaf", bufs=8))
            # p4 bufs=2 works because the adds are interleaved
            # p4_0,p4_1,p2a,p4_2,... so each reuse's reader precedes it
            # in the GPSIMD FIFO
            tpool = c2.enter_context(tc.tile_pool(name="tree", bufs=2))
            dpool = c2.enter_context(tc.tile_pool(name="den", bufs=1))
            dsbp = c2.enter_context(tc.tile_pool(name="dsb", bufs=2))

            def kproj_group(h, sc, kk_lo, kk_hi, kp):
                """Emit K-proj matmuls kk_lo..kk_hi for (head h, chunk sc)."""
                for kk in range(kk_lo, kk_hi):
                    nc.tensor.matmul(
                        kp[:], wk_sb[:, kk, h * 128:(h + 1) * 128],
                        y_sb[:, kk, sc * CW:(sc + 1) * CW],
                        start=(kk == 0), stop=(kk == KT - 1))

            def kproj_finish(h, sc, kp):
                dst = k_sb[:, h, sc * CW:(sc + 1) * CW]
                nc.vector.tensor_copy(dst, kp[:])
                _rope(nc, rope_pool, dst,
                      cosk_sb[:, sc * CW:(sc + 1) * CW],
                      sink_sb[:, sc * CW:(sc + 1) * CW], CW)

            # K proj head 0 upfront (drains on DVE; ACT idle)
            for sc in range(NCH):
                kp = pps.tile([128, CW], DT_F, tag="pp")
                kproj_group(0, sc, 0, KT, kp)
                kproj_finish(0, sc, kp)

            # normalize for (c, h) emitted one attention pass later: the
            # GPSIMD tree result p1 isn't ready when its own pass ends, so
            # the denominator matmul would stall PE if issued inline
            pending = []

            def emit_normalize():
                c, h, p1, av = pending.pop(0)
                den = pps.tile([1, CW], DT_F, tag="pp")
                nc.tensor.matmul(den[:], ones_col[:], p1[:],
                                 start=True, stop=True)
                den_sb = dsbp.tile([1, CW], DT_F, tag="dsb")
                nc.scalar.copy(den_sb[:], den[:])
                den_bc = dpool.tile([128, CW], DT_F, tag="dbc")
                nc.gpsimd.partition_broadcast(den_bc[:], den_sb[:])
                nc.vector.reciprocal_approx_fast(den_bc[:], den_bc[:])
                o_sb = opool.tile([128, CW], DT_B, tag="o")
                nc.vector.tensor_mul(o_sb[:], den_bc[:], av[:])
                o_tiles[(c, h)] = o_sb

            for h in range(HPC):
                for c in range(NCH):
                    # interleaved K proj (h+1, chunk c): 16 MMs woven into
                    # this attention pass, 2 per exp-paced gap
                    ikp = None
                    if h + 1 < HPC:
                        ikp = pps.tile([128, CW], DT_F, tag="pp")

                    q_ap = q_all[:, h, c * CW:(c + 1) * CW]
                    av = avps.tile([128, CW], DT_F, tag="av")
                    leaves = []
                    at_prev = None
                    for g in range(ST // 2):
                        sp = sps.tile([128, 2, CW], DT_F, tag="sp")
                        for j in range(2):
                            st = 2 * g + j
                            nc.tensor.matmul(
                                sp[:, j, :],
                                k_sb[:, h, st * 128:(st + 1) * 128],
                                q_ap, start=True, stop=True)
                        if ikp is not None:
                            kproj_group(h + 1, c, 2 * g, 2 * g + 2, ikp)
                        if at_prev is not None:
                            for j in range(2):
                                st = 2 * (g - 1) + j
                                nc.tensor.matmul(
                                    av[:],
                                    v_sb[:, st, h * 128:(h + 1) * 128],
                                    at_prev[:, j, :], start=(st == 0),
                                    stop=False)
                        if g == 6 and pending:
                            emit_normalize()
                        at = apool.tile([128, 2, CW], DT_B, tag="at")
                        nc.scalar.activation(
                            at[:, :, :], sp[:, :, :],
                            mybir.ActivationFunctionType.Exp, bias=zbias[:])
                        ps = lpool.tile([128, CW], DT_B, tag="ps")
                        nc.vector.tensor_add(ps[:], at[:, 0, :], at[:, 1, :])
                        leaves.append(ps)
                        at_prev = at
                    # tail: av for the last pair
                    for j in range(2):
                        st = ST - 2 + j
                        nc.tensor.matmul(
                            av[:], v_sb[:, st, h * 128:(h + 1) * 128],
                            at_prev[:, j, :], start=False,
                            stop=(st == ST - 1))
                    if ikp is not None:
                        kproj_finish(h + 1, c, ikp)

                    # denominator pair-sum tree on GPSIMD (otherwise idle);
                    # kv-partition reduction happens in the deferred
                    # normalize's ones-matmul
                    p4a = tpool.tile([128, CW], DT_B, tag="p4")
                    nc.gpsimd.tensor_add(p4a[:], leaves[0][:], leaves[1][:])
                    p4b = tpool.tile([128, CW], DT_B, tag="p4")
                    nc.gpsimd.tensor_add(p4b[:], leaves[2][:], leaves[3][:])
                    p2a = tpool.tile([128, CW], DT_B, tag="p2")
                    nc.gpsimd.tensor_add(p2a[:], p4a[:], p4b[:])
                    p4c = tpool.tile([128, CW], DT_B, tag="p4")
                    nc.gpsimd.tensor_add(p4c[:], leaves[4][:], leaves[5][:])
                    p4d = tpool.tile([128, CW], DT_B, tag="p4")
                    nc.gpsimd.tensor_add(p4d[:], leaves[6][:], leaves[7][:])
                    p2b = tpool.tile([128, CW], DT_B, tag="p2")
                    nc.gpsimd.tensor_add(p2b[:], p4c[:], p4d[:])
                    p1 = tpool.tile([128, CW], DT_B, tag="p1")
                    nc.gpsimd.tensor_add(p1[:], p2a[:], p2b[:])
                    pending.append((c, h, p1, av))

            while pending:
                emit_normalize()

        # ---- P3: output projection; drains split ACT/DVE; bf16 DMA ----
        with tc.tile_pool(name="part", bufs=3) as ppart:
            for c in range(NCH):
                for qt in range(CW // 128):
                    part_sb = ppart.tile([128, QD], DT_B, tag="part")
                    for nt in range(QD // 512):
                        fp = pps.tile([128, 512], DT_F, tag="pp")
                        for h in range(HPC):
                            nc.tensor.matmul(
                                fp[:],
                                o_tiles[(c, h)][:, qt * 128:(qt + 1) * 128],
                                wo_sb[:, h, nt * 512:(nt + 1) * 512],
                                start=(h == 0), stop=(h == HPC - 1))
                        if nt % 2 == 0:
                            nc.vector.tensor_copy(
                                part_sb[:, nt * 512:(nt + 1) * 512], fp[:])
                        else:
                            nc.scalar.copy(
                                part_sb[:, nt * 512:(nt + 1) * 512], fp[:])
                    row0 = c * CW + qt * 128
                    nc.sync.dma_start(partial[row0:row0 + 128, :],
                                      part_sb[:])


def _tile_rows(a, p=128):
    """[R, M] with R = n*p  ->  [p, n, M] (partition-major tiling)."""
    r, m = a.shape
    return np.ascontiguousarray(
        a.reshape(r // p, p, m).transpose(1, 0, 2))


def _host_shards(inputs):
    """Build the 8 per-core input maps from the full inputs."""
    x = np.asarray(inputs["x"], np.float32)
    y = np.asarray(inputs["y"], np.float32)
    rope_cos = np.asarray(inputs["rope_cos"], np.float32)
    rope_sin = np.asarray(inputs["rope_sin"], np.float32)
    wq = np.asarray(inputs["wq"], np.float32)
    wk = np.asarray(inputs["wk"], np.float32)
    wv = np.asarray(inputs["wv"], np.float32)
    wo = np.asarray(inputs["wo"], np.float32)
    input_pos = np.asarray(inputs["input_pos"], np.int64)

    # KV-cache scatter folded into a host-side permutation of y's rows and
    # of the rope tables (k positions live at cache slot input_pos[s]).
    y_cache = np.zeros((B, KVMAX, CD), np.float32)
    y_cache[:, input_pos, :] = y
    ck = np.zeros((KVMAX, NE // 2), np.float32)
    ck[input_pos] = rope_cos
    sk = np.zeros((KVMAX, NE // 2), np.float32)
    sk[input_pos] = rope_sin

    def tabT(cos2, sin2):
        cosT = np.tile(cos2.T, (2, 1)).astype(BF16)          # [NE, S]
        sinT = np.concatenate([-sin2.T, sin2.T], 0).astype(BF16)
        return np.ascontiguousarray(cosT), np.ascontiguousarray(sinT)

    cosq_h, sinq_h = tabT(rope_cos[:T], rope_sin[:T])
    cosk_h, sink_h = tabT(ck, sk)

    in_maps = []
    for core in range(N_CORES):
        b, hg = core // N_HG, core % N_HG
        rows = slice(hg * HD, (hg + 1) * HD)
        xt = _tile_rows(x[b].T.astype(BF16))        # [128, KT, T]
        xt = np.ascontiguousarray(
            xt.reshape(128, KT, NCH, CW).transpose(0, 2, 1, 3))
        in_maps.append({
            "xT": xt,                               # [128, NCH, KT, CW]
            "yT": _tile_rows(y_cache[b].T.astype(BF16)),
            "wqT": _tile_rows((wq[rows] * SCALE).T.astype(BF16)),
            "wkT": _tile_rows(wk[rows].T.astype(BF16)),
            "wvT": _tile_rows(wv[rows].T.astype(BF16)),
            "woT": _tile_rows(wo[:, rows].T.astype(BF16)),
            "cosq": cosq_h, "sinq": sinq_h,
            "cosk": cosk_h, "sink": sink_h,
        })
    return in_maps


def _run(inputs, trace=False, reps=1, **kw):
    nc = _build(reps)
    in_maps = _host_shards(inputs)
    res = run_bass_kernel_spmd(nc, in_maps, list(range(N_CORES)),
                               trace=trace, **kw)
    out = np.zeros((B, T, QD), np.float32)
    for core in range(N_CORES):
        out[core // N_HG] += np.asarray(res.results[core]["partial"],
                                        dtype=np.float32)
    return out, res


def kernel(**inputs):
    out, _ = _run(inputs)
    return out


# revision 19
# speedup vs baseline: 2.3031x; 1.1884x over previous
"""Cross-attention Trainium2 kernel, tensor-parallel over 8 NeuronCores.

Sharding: core c handles batch b = c // 4 and head-group hg = c % 4
(4 heads = 512 of the 2048 hidden dims). Each core computes its heads'
QKV projections, RoPE, softmax attention (transposed-scores layout), and
a partial output projection. The host sums the 4 partials per batch.

Schedule (PE-roofline oriented):
  P0  all Q projections + Q RoPE, overlapped with every input DMA
  P1  V projection (PSUM drains on ACT, which is otherwise idle)
  P2  per head h: attention over the 4 q-chunks with the NEXT head's
      K-projection matmuls interleaved into the exp-paced gaps; softmax
      denominator via DVE pair sums + lagged ones-matmuls on PE;
      normalize (GPSIMD broadcast + reciprocal_approx_fast + mul)
      deferred one pass so its latency never stalls PE
  P3  output projection, drains split ACT/DVE, bf16 partial DMA

Self-contained: hardcodes all shapes from the problem spec.
"""

import numpy as np
import ml_dtypes

import concourse.bacc as bacc
import concourse.bass_isa as bass_isa
import concourse.tile as tile
from concourse import mybir
from concourse.bass_utils import run_bass_kernel_spmd

BF16 = ml_dtypes.bfloat16

B, T, S = 2, 2048, 2048
QD, CD = 2048, 2048
H, D = 16, 128
NE = 64            # rotary dims
KVMAX = 2048
N_CORES = 8
N_HG = 4           # head groups (cores per batch)
HPC = H // N_HG    # heads per core = 4
HD = HPC * D       # 512 head dims per core
KT = CD // 128     # contraction tiles = 16
ST = KVMAX // 128  # kv tiles = 16
CW = 512           # q chunk width
NCH = T // CW      # 4 chunks
SCALE = float(D) ** -0.5
IDENT32 = list(range(32))

DT_B = mybir.dt.bfloat16
DT_F = mybir.dt.float32

_compiled = {}


def _build(reps=1):
    if reps in _compiled:
        return _compiled[reps]

    nc = bacc.Bacc("TRN2", target_bir_lowering=False, debug=False,
                   num_devices=N_CORES)

    xT = nc.dram_tensor("xT", [128, NCH, KT, CW], DT_B, kind="ExternalInput")
    yT = nc.dram_tensor("yT", [128, KT, KVMAX], DT_B, kind="ExternalInput")
    wqT = nc.dram_tensor("wqT", [128, KT, HD], DT_B, kind="ExternalInput")
    wkT = nc.dram_tensor("wkT", [128, KT, HD], DT_B, kind="ExternalInput")
    wvT = nc.dram_tensor("wvT", [128, KT, HD], DT_B, kind="ExternalInput")
    woT = nc.dram_tensor("woT", [128, HPC, QD], DT_B, kind="ExternalInput")
    cosq = nc.dram_tensor("cosq", [NE, T], DT_B, kind="ExternalInput")
    sinq = nc.dram_tensor("sinq", [NE, T], DT_B, kind="ExternalInput")
    cosk = nc.dram_tensor("cosk", [NE, KVMAX], DT_B, kind="ExternalInput")
    sink = nc.dram_tensor("sink", [NE, KVMAX], DT_B, kind="ExternalInput")
    partial = nc.dram_tensor("partial", [T, QD], DT_B, kind="ExternalOutput")

    with tile.TileContext(nc) as tc:
        if reps == 1:
            _body(nc, tc, xT, yT, wqT, wkT, wvT, woT, cosq, sinq, cosk,
                  sink, partial)
        else:
            with tc.For_i(0, reps, 1):
                _body(nc, tc, xT, yT, wqT, wkT, wvT, woT, cosq, sinq,
                      cosk, sink, partial)

    nc.compile()
    _compiled[reps] = nc
    return nc


def _rope(nc, pool, dst, cos_sb, sin_sb, w):
    """In-place RoPE on dst[0:NE, :w] (head-dim on partitions).

    cos_sb/sin_sb are [NE, w] slices; sin rows 0:32 carry -sin, 32:64 +sin.
    """
    rot = pool.tile([NE, w], DT_B, tag="rot")
    half = NE // 2
    nc.vector.stream_shuffle(rot[0:half, :], dst[half:NE, :], IDENT32)
    nc.vector.stream_shuffle(rot[half:NE, :], dst[0:half, :], IDENT32)
    nc.vector.tensor_mul(rot[:, :], rot[:, :], sin_sb)
    nc.vector.tensor_mul(dst[0:NE, :], dst[0:NE, :], cos_sb)
    nc.vector.tensor_add(dst[0:NE, :], dst[0:NE, :], rot[:, :])


def _body(nc, tc, xT, yT, wqT, wkT, wvT, woT, cosq, sinq, cosk, sink,
          partial):
    from contextlib import ExitStack

    with ExitStack() as ctx:
        const = ctx.enter_context(tc.tile_pool(name="const", bufs=1))
        qpool = ctx.enter_context(tc.tile_pool(name="q", bufs=1))
        kvpool = ctx.enter_context(tc.tile_pool(name="kv", bufs=1))
        ktab = ctx.enter_context(tc.tile_pool(name="ktab", bufs=1))
        rope_pool = ctx.enter_context(tc.tile_pool(name="rope", bufs=2))
        # PSUM: pps 2 + sps 2x2 + avps 2 = 8 banks
        pps = ctx.enter_context(tc.tile_pool(name="pps", bufs=2,
                                             space="PSUM"))
        sps = ctx.enter_context(tc.tile_pool(name="sps", bufs=2,
                                             space="PSUM"))
        avps = ctx.enter_context(tc.tile_pool(name="avps", bufs=2,
                                              space="PSUM"))

        zbias = const.tile([128, 1], DT_F)
        nc.gpsimd.memset(zbias[:], 0.0)
        ones_col = const.tile([128, 1], DT_B)
        nc.gpsimd.memset(ones_col[:], 1.0)

        q_all = qpool.tile([128, HPC, T], DT_B)
        k_sb = kvpool.tile([128, HPC, KVMAX], DT_B)
        v_sb = kvpool.tile([128, ST, HD], DT_B)
        cosk_sb = ktab.tile([NE, KVMAX], DT_B)
        sink_sb = ktab.tile([NE, KVMAX], DT_B)
        nc.sync.dma_start(cosk_sb[:], cosk.ap())
        nc.sync.dma_start(sink_sb[:], sink.ap())

        # ---- P0: all Q projections (+ RoPE) while x streams ----
        with ExitStack() as c0:
            wqpool = c0.enter_context(tc.tile_pool(name="wq", bufs=1))
            xpool = c0.enter_context(tc.tile_pool(name="x", bufs=2))
            qtab = c0.enter_context(tc.tile_pool(name="qtab", bufs=1))

            wq_sb = wqpool.tile([128, KT, HD], DT_B)
            for g in range(4):
                nc.sync.dma_start(wq_sb[:, g * 4:(g + 1) * 4, :],
                                  wqT.ap()[:, g * 4:(g + 1) * 4, :])
            cosq_sb = qtab.tile([NE, T], DT_B)
            nc.sync.dma_start(cosq_sb[:], cosq.ap())
            sinq_sb = qtab.tile([NE, T], DT_B)
            nc.sync.dma_start(sinq_sb[:], sinq.ap())

            x_tiles = []
            for c in range(NCH):
                x_sb = xpool.tile([128, KT, CW], DT_B, tag="x")
                for g in range(4):
                    nc.sync.dma_start(x_sb[:, g * 4:(g + 1) * 4, :],
                                      xT.ap()[:, c, g * 4:(g + 1) * 4, :])
                x_tiles.append(x_sb)

            for ci in range(NCH):
                x_sb = x_tiles[ci]
                for h in range(HPC):
                    qp = pps.tile([128, CW], DT_F, tag="pp")
                    for kk in range(KT):
                        nc.tensor.matmul(
                            qp[:], wq_sb[:, kk, h * 128:(h + 1) * 128],
                            x_sb[:, kk, :],
                            start=(kk == 0), stop=(kk == KT - 1))
                    dst = q_all[:, h, ci * CW:(ci + 1) * CW]
                    if h % 2 == 0:
                        nc.vector.tensor_copy(dst, qp[:])
                    else:
                        nc.scalar.copy(dst, qp[:])
                    _rope(nc, rope_pool, dst,
                          cosq_sb[:, ci * CW:(ci + 1) * CW],
                          sinq_sb[:, ci * CW:(ci + 1) * CW], CW)

        # y / wk / wv / wo loads issued behind the P0 x DMAs
        ypool = ctx.enter_context(tc.tile_pool(name="y", bufs=1))
        wkpool = ctx.enter_context(tc.tile_pool(name="wk", bufs=1))
        y_sb = ypool.tile([128, KT, KVMAX], DT_B)
        for g in range(8):
            nc.sync.dma_start(y_sb[:, g * 2:(g + 1) * 2, :],
                              yT.ap()[:, g * 2:(g + 1) * 2, :])
        wk_sb = wkpool.tile([128, KT, HD], DT_B)
        for g in range(4):
            nc.sync.dma_start(wk_sb[:, g * 4:(g + 1) * 4, :],
                              wkT.ap()[:, g * 4:(g + 1) * 4, :])

        with ExitStack() as c1:
            wvpool = c1.enter_context(tc.tile_pool(name="wv", bufs=1))
            wv_sb = wvpool.tile([128, KT, HD], DT_B)
            for g in range(4):
                nc.sync.dma_start(wv_sb[:, g * 4:(g + 1) * 4, :],
                                  wvT.ap()[:, g * 4:(g + 1) * 4, :])

            # ---- P1: V projection; drains on ACT (idle here) ----
            for st in range(ST):
                vp = pps.tile([128, HD], DT_F, tag="pp")
                for kk in range(KT):
                    nc.tensor.matmul(
                        vp[:], y_sb[:, kk, st * 128:(st + 1) * 128],
                        wv_sb[:, kk, :],
                        start=(kk == 0), stop=(kk == KT - 1))
                nc.scalar.copy(v_sb[:, st, :], vp[:])

        # ---- P2: per-head attention with next head's K proj woven in ----
        o_tiles = {}
        opool = ctx.enter_context(tc.tile_pool(name="o", bufs=16))
        wopool = ctx.enter_context(tc.tile_pool(name="wo", bufs=1))
        wo_sb = wopool.tile([128, HPC, QD], DT_B)
        for g in range(HPC):
            nc.sync.dma_start(wo_sb[:, g, :], woT.ap()[:, g, :])
        with ExitStack() as c2:
            apool = c2.enter_context(tc.tile_pool(name="attn", bufs=3))
            lpool = c2.enter_context(tc.tile_pool(name="leaf", bufs=8))
            dpool = c2.enter_context(tc.tile_pool(name="den", bufs=1))
            dsbp = c2.enter_context(tc.tile_pool(name="dsb", bufs=2))

            def kproj_group(h, sc, kk_lo, kk_hi, kp):
                """Emit K-proj matmuls kk_lo..kk_hi for (head h, chunk sc)."""
                for kk in range(kk_lo, kk_hi):
                    nc.tensor.matmul(
                        kp[:], wk_sb[:, kk, h * 128:(h + 1) * 128],
                        y_sb[:, kk, sc * CW:(sc + 1) * CW],
                        start=(kk == 0), stop=(kk == KT - 1))

            def kproj_finish(h, sc, kp):
                dst = k_sb[:, h, sc * CW:(sc + 1) * CW]
                nc.vector.tensor_copy(dst, kp[:])
                _rope(nc, rope_pool, dst,
                      cosk_sb[:, sc * CW:(sc + 1) * CW],
                      sink_sb[:, sc * CW:(sc + 1) * CW], CW)

            # K proj head 0 upfront (drains on DVE; ACT idle)
            for sc in range(NCH):
                kp = pps.tile([128, CW], DT_F, tag="pp")
                kproj_group(0, sc, 0, KT, kp)
                kproj_finish(0, sc, kp)

            # normalize for (c, h) emitted one attention pass later: the
            # GPSIMD tree result p1 isn't ready when its own pass ends, so
            # the denominator matmul would stall PE if issued inline
            pending = []

            def emit_normalize():
                c, h, den_sb, av = pending.pop(0)
                den_bc = dpool.tile([128, CW], DT_F, tag="dbc")
                nc.gpsimd.partition_broadcast(den_bc[:], den_sb[:])
                nc.vector.reciprocal_approx_fast(den_bc[:], den_bc[:])
                o_sb = opool.tile([128, CW], DT_B, tag="o")
                nc.vector.tensor_mul(o_sb[:], den_bc[:], av[:])
                o_tiles[(c, h)] = o_sb

            for h in range(HPC):
                for c in range(NCH):
                    # interleaved K proj (h+1, chunk c): 16 MMs woven into
                    # this attention pass, 2 per exp-paced gap
                    ikp = None
                    if h + 1 < HPC:
                        ikp = pps.tile([128, CW], DT_F, tag="pp")

                    q_ap = q_all[:, h, c * CW:(c + 1) * CW]
                    av = avps.tile([128, CW], DT_F, tag="av")
                    den = pps.tile([1, CW], DT_F, tag="pp")
                    leaves = []
                    at_prev = None
                    for g in range(ST // 2):
                        sp = sps.tile([128, 2, CW], DT_F, tag="sp")
                        for j in range(2):
                            st = 2 * g + j
                            nc.tensor.matmul(
                                sp[:, j, :],
                                k_sb[:, h, st * 128:(st + 1) * 128],
                                q_ap, start=True, stop=True)
                        if ikp is not None:
                            kproj_group(h + 1, c, 2 * g, 2 * g + 2, ikp)
                        if at_prev is not None:
                            for j in range(2):
                                st = 2 * (g - 1) + j
                                nc.tensor.matmul(
                                    av[:],
                                    v_sb[:, st, h * 128:(h + 1) * 128],
                                    at_prev[:, j, :], start=(st == 0),
                                    stop=False)
                        if g == 6 and pending:
                            emit_normalize()
                        at = apool.tile([128, 2, CW], DT_B, tag="at")
                        nc.scalar.activation(
                            at[:, :, :], sp[:, :, :],
                            mybir.ActivationFunctionType.Exp, bias=zbias[:])
                        ps = lpool.tile([128, CW], DT_B, tag="ps")
                        nc.vector.tensor_add(ps[:], at[:, 0, :], at[:, 1, :])
                        # den matmul lags its leaf by 3 groups so the DVE
                        # queue (leaf adds behind rope bursts) never stalls PE
                        if len(leaves) >= 3:
                            lp = leaves[len(leaves) - 3]
                            nc.tensor.matmul(den[:], ones_col[:], lp[:],
                                             start=(len(leaves) == 3),
                                             stop=False)
                        leaves.append(ps)
                        at_prev = at
                    # tail: av for the last pair, then the lagged den MMs —
                    # all before kproj_finish's DVE burst
                    for j in range(2):
                        st = ST - 2 + j
                        nc.tensor.matmul(
                            av[:], v_sb[:, st, h * 128:(h + 1) * 128],
                            at_prev[:, j, :], start=False,
                            stop=(st == ST - 1))
                    for i in (5, 6, 7):
                        nc.tensor.matmul(den[:], ones_col[:], leaves[i][:],
                                         start=False, stop=(i == 7))
                    den_sb = dsbp.tile([1, CW], DT_F, tag="dsb")
                    nc.scalar.copy(den_sb[:], den[:])
                    if ikp is not None:
                        kproj_finish(h + 1, c, ikp)
                    pending.append((c, h, den_sb, av))

            while pending:
                emit_normalize()

        # ---- P3: output projection; drains split ACT/DVE; bf16 DMA ----
        with tc.tile_pool(name="part", bufs=3) as ppart:
            for c in range(NCH):
                for qt in range(CW // 128):
                    part_sb = ppart.tile([128, QD], DT_B, tag="part")
                    for nt in range(QD // 512):
                        fp = pps.tile([128, 512], DT_F, tag="pp")
                        for h in range(HPC):
                            nc.tensor.matmul(
                                fp[:],
                                o_tiles[(c, h)][:, qt * 128:(qt + 1) * 128],
                                wo_sb[:, h, nt * 512:(nt + 1) * 512],
                                start=(h == 0), stop=(h == HPC - 1))
                        if nt % 2 == 0:
                            nc.vector.tensor_copy(
                                part_sb[:, nt * 512:(nt + 1) * 512], fp[:])
                        else:
                            nc.scalar.copy(
                                part_sb[:, nt * 512:(nt + 1) * 512], fp[:])
                    row0 = c * CW + qt * 128
                    nc.sync.dma_start(partial[row0:row0 + 128, :],
                                      part_sb[:])


def _tile_rows(a, p=128):
    """[R, M] with R = n*p  ->  [p, n, M] (partition-major tiling)."""
    r, m = a.shape
    return np.ascontiguousarray(
        a.reshape(r // p, p, m).transpose(1, 0, 2))


def _host_shards(inputs):
    """Build the 8 per-core input maps from the full inputs."""
    x = np.asarray(inputs["x"], np.float32)
    y = np.asarray(inputs["y"], np.float32)
    rope_cos = np.asarray(inputs["rope_cos"], np.float32)
    rope_sin = np.asarray(inputs["rope_sin"], np.float32)
    wq = np.asarray(inputs["wq"], np.float32)
    wk = np.asarray(inputs["wk"], np.float32)
    wv = np.asarray(inputs["wv"], np.float32)
    wo = np.asarray(inputs["wo"], np.float32)
    input_pos = np.asarray(inputs["input_pos"], np.int64)

    # KV-cache scatter folded into a host-side permutation of y's rows and
    # of the rope tables (k positions live at cache slot input_pos[s]).
    y_cache = np.zeros((B, KVMAX, CD), np.float32)
    y_cache[:, input_pos, :] = y
    ck = np.zeros((KVMAX, NE // 2), np.float32)
    ck[input_pos] = rope_cos
    sk = np.zeros((KVMAX, NE // 2), np.float32)
    sk[input_pos] = rope_sin

    def tabT(cos2, sin2):
        cosT = np.tile(cos2.T, (2, 1)).astype(BF16)          # [NE, S]
        sinT = np.concatenate([-sin2.T, sin2.T], 0).astype(BF16)
        return np.ascontiguousarray(cosT), np.ascontiguousarray(sinT)

    cosq_h, sinq_h = tabT(rope_cos[:T], rope_sin[:T])
    cosk_h, sink_h = tabT(ck, sk)

    in_maps = []
    for core in range(N_CORES):
        b, hg = core // N_HG, core % N_HG
        rows = slice(hg * HD, (hg + 1) * HD)
        xt = _tile_rows(x[b].T.astype(BF16))        # [128, KT, T]
        xt = np.ascontiguousarray(
            xt.reshape(128, KT, NCH, CW).transpose(0, 2, 1, 3))
        in_maps.append({
            "xT": xt,                               # [128, NCH, KT, CW]
            "yT": _tile_rows(y_cache[b].T.astype(BF16)),
            "wqT": _tile_rows((wq[rows] * SCALE).T.astype(BF16)),
            "wkT": _tile_rows(wk[rows].T.astype(BF16)),
            "wvT": _tile_rows(wv[rows].T.astype(BF16)),
            "woT": _tile_rows(wo[:, rows].T.astype(BF16)),
            "cosq": cosq_h, "sinq": sinq_h,
            "cosk": cosk_h, "sink": sink_h,
        })
    return in_maps


def _run(inputs, trace=False, reps=1, **kw):
    nc = _build(reps)
    in_maps = _host_shards(inputs)
    res = run_bass_kernel_spmd(nc, in_maps, list(range(N_CORES)),
                               trace=trace, **kw)
    out = np.zeros((B, T, QD), np.float32)
    for core in range(N_CORES):
        out[core // N_HG] += np.asarray(res.results[core]["partial"],
                                        dtype=np.float32)
    return out, res


def kernel(**inputs):
    out, _ = _run(inputs)
    return out
